# revision 1
# baseline (speedup 1.0000x reference)
"""Trainium2 Bass kernel for nn_Aligner — v2 (restructured critical path).

Sharding identical to baseline (tensor-parallel LSTM gates + batch-parallel
attention, 2 AllGathers/step). v2 restructures the per-step chain:
  - attention argument split: statics (pm copy + G x im2col) issue before the
    h AllGather lands; only the pq x indicator matmuls wait on h.
  - gate matmul split: bias+h-part accumulates into PSUM during the w
    AllGather; only the 4 w-part matmuls are on the chain.
  - LSTM pointwise fused: g-gate weights pre-scaled by 2 on host so one
    tanh(0.5*gates) activation covers i,f,o,g; scalar_tensor_tensor fusions.
  - pq / out-projection run in f32r off hTg/wT directly (hTb/wTb copies
    dropped).
"""
import json
import os

import numpy as np
import ml_dtypes

import concourse.bass as bass
import concourse.mybir as mybir
import concourse.tile as tile

B, T, D = 64, 512, 512
RNN, ATT, OUT = 1024, 128, 80
SPK_DIM, N_SPKRS = 64, 128
NF, KS = 32, 31
PAD = (KS - 1) // 2
NC_N = 8
BL = B // NC_N            # 8 samples/core
GS = 4 * RNN // NC_N      # 512 gate cols/core
BT = BL * T               # 4096
TPAD = T + 32             # 544 padded alpha row stride
NQ = 4                    # attention quarters
QW = BT // NQ             # 1024

f32 = mybir.dt.float32
bf16 = mybir.dt.bfloat16
f32r = mybir.dt.float32r
AF = mybir.ActivationFunctionType
ALU = mybir.AluOpType

_CACHE = {}


def _fix_bir_json(bir):
    """This walrus build allows at most one semaphore wait per instruction;
    hoist extras onto same-engine NoOps inserted just before."""
    j = json.loads(bir)
    n = [0]

    def fix_block(block):
        insts = block.get("instructions")
        if not insts:
            return
        out = []
        for ins in insts:
            waits = (ins.get("sync_info") or {}).get("on_wait") or []
            if len(waits) > 1:
                ins["sync_info"]["on_wait"] = waits[-1:]
                for w in waits[:-1]:
                    n[0] += 1
                    out.append({"engine": ins["engine"], "ins": [], "outs": [],
                                "name": f"I-mwfix-{n[0]}", "opcode": "NoOp",
                                "sync_info": {"on_wait": [w]}})
            out.append(ins)
        block["instructions"] = out

    def walk(o):
        if isinstance(o, dict):
            if isinstance(o.get("instructions"), list):
                fix_block(o)
            for v in o.values():
                walk(v)
        elif isinstance(o, list):
            for v in o:
                walk(v)

    walk(j)
    return json.dumps(j).encode()


def _install_hooks():
    if getattr(_install_hooks, "done", False):
        return
    _install_hooks.done = True
    import concourse.bass_utils as bu
    import concourse.bass2jax as b2j
    orig = bu.compile_bir_kernel

    def patched(bir_json, tmpdir, neff_name="file.neff"):
        if isinstance(bir_json, str):
            bir_json = bir_json.encode()
        return orig(_fix_bir_json(bir_json), tmpdir, neff_name=neff_name)

    bu.compile_bir_kernel = patched
    b2j.compile_bir_kernel = patched


def _build(n_steps):
    SKIP_CC = os.environ.get("SKIP_CC", "0") == "1"

    nc = bass.Bass("TRN2", target_bir_lowering=False, debug=False,
                   num_devices=NC_N)

    def din(name, shape, dt=f32):
        return nc.dram_tensor(name, shape, dt, kind="ExternalInput")

    x_wg = din("wg", [12 * 128, GS], bf16)
    x_b0 = din("b0", [B, GS], bf16)
    x_pm = din("pm", [ATT, BT], bf16)
    x_g = din("gmat", [62, ATT], bf16)
    x_ind = din("indic", [B, BT], bf16)     # rows: global sample one-hots
    x_idf = din("identf", [128, 128])
    x_idb = din("identb", [128, 128], bf16)
    x_wv = din("wvoh", [ATT, 8 * BL], bf16)
    x_wq = din("wqt", [RNN, ATT], bf16)
    x_wp = din("wpt", [12 * 128, OUT], bf16)
    x_bp = din("bpv", [1, OUT], bf16)
    x_on = din("onesb", [1, B], bf16)
    x_mm = din("memt", [128, BL * 4 * D], bf16)
    out_d = nc.dram_tensor("out", [B, n_steps, OUT], f32, kind="ExternalOutput")

    RG = [list(range(NC_N))]

    with tile.TileContext(nc) as tc:
        with (
            tc.tile_pool(name="const", bufs=1) as cst,
            tc.tile_pool(name="state", bufs=1) as st,
            tc.tile_pool(name="work", bufs=2) as wk,
            tc.tile_pool(name="psA", bufs=2, space="PSUM") as psA,
            tc.tile_pool(name="psB", bufs=1, space="PSUM") as psB,
            tc.tile_pool(name="dram", bufs=1, space="DRAM") as dram,
        ):
            # ---------------- constants
            wg = cst.tile([128, 12 * GS], bf16)
            for c in range(12):
                nc.sync.dma_start(wg[:, c * GS:(c + 1) * GS],
                                  x_wg[c * 128:(c + 1) * 128, :])
            b0 = cst.tile([B, GS], bf16)
            nc.sync.dma_start(b0[:], x_b0[:])
            pm = cst.tile([ATT, BT], bf16)
            nc.sync.dma_start(pm[:], x_pm[:])
            idf = cst.tile([128, 128], f32)
            nc.sync.dma_start(idf[:], x_idf[:])
            idr = cst.tile([128, 128], f32r)
            nc.sync.dma_start(idr[:], x_idf[:].bitcast(f32r))
            idb = cst.tile([128, 128], bf16)
            nc.sync.dma_start(idb[:], x_idb[:])
            wv = cst.tile([ATT, 8 * BL], bf16)
            nc.sync.dma_start(wv[:], x_wv[:])
            wq = cst.tile([128, 8 * ATT], bf16)
            for c in range(8):
                nc.sync.dma_start(wq[:, c * ATT:(c + 1) * ATT],
                                  x_wq[c * 128:(c + 1) * 128, :])
            wp = cst.tile([128, 12 * OUT], bf16)
            for c in range(12):
                nc.sync.dma_start(wp[:, c * OUT:(c + 1) * OUT],
                                  x_wp[c * 128:(c + 1) * 128, :])
            bpv = cst.tile([1, OUT], bf16)
            nc.sync.dma_start(bpv[:], x_bp[:])
            onb = cst.tile([1, B], bf16)
            nc.sync.dma_start(onb[:], x_on[:])
            mm = cst.tile([128, BL * 4 * D], bf16)
            nc.sync.dma_start(mm[:], x_mm[:])

            # ---------------- state (hTg/wT double-buffered by step parity)
            hTg_bufs = [st.tile([128, 8 * B], bf16, name=f"hTg{i}")
                        for i in range(2)]
            wT_bufs = [st.tile([128, 4 * B], bf16, name=f"wT{i}")
                       for i in range(2)]
            c_st = st.tile([B, 128], f32)
            a_cum = st.tile([BL, T], f32)
            af = st.tile([2, BL * TPAD], bf16)
            awm = st.tile([128, 32 * 32], bf16)
            th = st.tile([ATT, BT], bf16)
            # fused conv operands: lhsT rows 0:64 pq (dynamic), 64:126 G
            conv_lhs = st.tile([126, ATT], bf16)
            nc.sync.dma_start(conv_lhs[B:126, :], x_g[:])
            conv_rhs = st.tile([126, BT], bf16)
            nc.sync.dma_start(conv_rhs[0:B, :], x_ind[:])
            for t_ in (c_st, a_cum, af, awm):
                nc.vector.memset(t_[:], 0.0)
            nc.vector.memset(th[:], 0.0)
            for t_ in hTg_bufs + wT_bufs:
                nc.vector.memset(t_[:], 0.0)
            nc.vector.memset(conv_rhs[B:126, :], 0.0)
            nc.vector.memset(conv_lhs[0:B, :], 0.0)

            # ---------------- collective bounce buffers
            bh_in = dram.tile([128, B], bf16)
            bh_out = dram.tile([NC_N * 128, B], bf16)
            bw_in = dram.tile([BL, D], bf16)
            bw_out = dram.tile([B, D], bf16)
            baf = dram.tile([2, BL * TPAD], bf16)

            # ---------------- steps
            for t in range(n_steps):
                hTg_p = hTg_bufs[(t + 1) % 2]   # h(t-1) gathered
                hTg = hTg_bufs[t % 2]           # h(t) gathered (this step)
                wT_p = wT_bufs[(t + 1) % 2]     # w(t-1) gathered
                wT = wT_bufs[t % 2]             # w(t) gathered (this step)
                # ---- gates: bias + h-part early (runs during AG-w of t-1)
                ps_g = psB.tile([B, GS], f32, tag="gates")
                nc.tensor.matmul(ps_g[:], idb[0:B, 0:B], b0[:],
                                 start=True, stop=False)
                for c in range(8):
                    nc.tensor.matmul(ps_g[:], hTg_p[:, c * B:(c + 1) * B],
                                     wg[:, (4 + c) * GS:(5 + c) * GS],
                                     start=False, stop=False)
                # ---- gates: w-part (chain: waits on wT(t-1))
                for c in range(4):
                    nc.tensor.matmul(ps_g[:], wT_p[:, c * B:(c + 1) * B],
                                     wg[:, c * GS:(c + 1) * GS],
                                     start=False, stop=(c == 3))

                # ---- pointwise LSTM (g-weights pre-scaled 2x on host):
                # yifog = tanh(0.5*gates): cols 0:384 -> 2*sig(x)-1, 384:512 tanh(g)
                yifog = wk.tile([B, 512], f32, tag="yifog")
                nc.scalar.activation(yifog[:], ps_g[:], AF.Tanh, scale=0.5)
                t1 = wk.tile([B, 128], f32, tag="t1")
                t2 = wk.tile([B, 128], f32, tag="t2")
                # t1 = (f'+1)*c ; t2 = (i'+1)*tanh(g) ; t1 = t1+t2 = 2*c_new
                nc.vector.scalar_tensor_tensor(
                    t1[:], yifog[:, 128:256], 1.0, c_st[:], ALU.add, ALU.mult)
                nc.vector.scalar_tensor_tensor(
                    t2[:], yifog[:, 0:128], 1.0, yifog[:, 384:512],
                    ALU.add, ALU.mult)
                nc.vector.tensor_add(t1[:], t1[:], t2[:])
                tct = wk.tile([B, 128], f32, tag="tct")
                nc.scalar.activation(tct[:], t1[:], AF.Tanh, scale=0.5)
                h2 = wk.tile([B, 128], bf16, tag="h2")
                nc.vector.scalar_tensor_tensor(
                    h2[:], yifog[:, 256:384], 1.0, tct[:], ALU.add, ALU.mult)
                nc.vector.tensor_scalar_mul(c_st[:], t1[:], 0.5)  # off-chain

                # ---- h2 -> transpose -> DMA -> AllGather -> hTg
                ps_t = psB.tile([128, 256], bf16, tag="misc")
                nc.tensor.transpose(ps_t[:, 0:B], h2[:], idb[0:B, 0:B])
                hto = wk.tile([128, B], bf16, tag="hto")
                nc.vector.tensor_copy(hto[:], ps_t[:, 0:B])
                nc.sync.dma_start(bh_in[:], hto[:])
                if not SKIP_CC:
                    nc.gpsimd.collective_compute(
                        "AllGather", ALU.bypass, ins=[bh_in.opt()],
                        outs=[bh_out.opt()], replica_groups=RG)
                else:
                    for cc in range(8):
                        nc.sync.dma_start(bh_out[cc * 128:(cc + 1) * 128, :],
                                          bh_in[:])
                bho = bh_out[:, :]
                srch = bass.AP(tensor=bho.tensor, offset=bho.offset,
                               ap=[[B, 128], [128 * B, 8], [1, B]])
                nc.sync.dma_start(hTg[:, :], srch)

                # ---- pq (all 64 samples; f32r) -> bf16 lhsT for dyn MMs
                ps_pq = psB.tile([B, ATT], f32, tag="misc")
                for c in range(8):
                    nc.tensor.matmul(ps_pq[:],
                                     hTg[:, c * B:(c + 1) * B],
                                     wq[:, c * ATT:(c + 1) * ATT],
                                     start=(c == 0), stop=(c == 7))
                nc.vector.tensor_copy(conv_lhs[0:B, :], ps_pq[:])

                # ---- attention dynamics: pq x indicator, tanh, e
                ps_e = psB.tile([8, 512], f32, tag="e")
                for q in range(NQ):
                    ps_q = psA.tile([ATT, QW], f32, tag="argq")
                    for c in range(QW // 512):
                        lo = c * 512
                        g_lo = q * QW + lo
                        nc.tensor.matmul(ps_q[:, lo:lo + 512], idb[:, 0:ATT],
                                         pm[:, g_lo:g_lo + 512],
                                         start=True, stop=False)
                        nc.tensor.matmul(ps_q[:, lo:lo + 512], conv_lhs[:],
                                         conv_rhs[:, g_lo:g_lo + 512],
                                         start=False, stop=True)
                    nc.scalar.activation(th[:, q * QW:(q + 1) * QW], ps_q[:],
                                         AF.Tanh)
                    for bl in (2 * q, 2 * q + 1):
                        nc.tensor.matmul(ps_e[:], wv[:, bl * BL:(bl + 1) * BL],
                                         th[:, bl * T:(bl + 1) * T],
                                         start=(bl == 0), stop=(bl == 7))

                # ---- softmax (unnormalized exp + folded normalization)
                aw_b = wk.tile([BL, T], bf16, tag="awb")
                s_t = wk.tile([BL, 1], f32, tag="s")
                nc.scalar.activation(aw_b[:], ps_e[:], AF.Exp, accum_out=s_t[:])
                rs = wk.tile([BL, 1], f32, tag="rs")
                nc.vector.reciprocal(rs[:], s_t[:])
                aw_n = wk.tile([BL, T], bf16, tag="awn")
                nc.vector.tensor_scalar_mul(aw_n[:], aw_b[:], rs[:])
                nc.vector.scalar_tensor_tensor(
                    a_cum[:], aw_b[:], rs[:], a_cum[:], ALU.mult, ALU.add)
                acb = wk.tile([BL, T], bf16, tag="acb")
                nc.vector.tensor_copy(acb[:], a_cum[:])

                # ---- alpha-flat rows + im2col for next step's conv statics
                af_r = af[:, :].rearrange("c (b p) -> c b p", b=BL)
                nc.scalar.dma_start(af_r[0:1, :, 16:16 + T], aw_n[:])
                nc.scalar.dma_start(af_r[1:2, :, 16:16 + T], acb[:])
                nc.gpsimd.dma_start(baf[:], af[:])
                baf_ap = baf[:, :]
                for ci in range(2):
                    for hh in range(2):
                        k0 = hh * 16
                        nk = 16 if hh == 0 else KS - 16
                        src = bass.AP(
                            tensor=baf_ap.tensor,
                            offset=baf_ap.offset + ci * BL * TPAD + k0 + 1,
                            ap=[[1, nk], [TPAD, BL], [1, T]])
                        nc.gpsimd.dma_start(
                            conv_rhs[B + ci * KS + k0:
                                     B + ci * KS + k0 + nk, :], src)

                # ---- aw transposes -> masked diagonal blocks for w_new
                ps_at = psB.tile([128, 256], bf16, tag="misc2")
                for tc_i in range(4):
                    nc.tensor.transpose(ps_at[:, tc_i * BL:(tc_i + 1) * BL],
                                        aw_n[:, tc_i * 128:(tc_i + 1) * 128],
                                        idb[0:BL, 0:BL])
                awm_ap = awm[:, :]
                pa_ap = ps_at[:, 0:32]
                dst = bass.AP(tensor=awm_ap.tensor, offset=awm_ap.offset,
                              ap=[[awm_ap.ap[0][0], 128], [256, 4], [129, 2],
                                  [32, 4]])
                src = bass.AP(tensor=pa_ap.tensor, offset=pa_ap.offset,
                              ap=[[pa_ap.ap[0][0], 128], [1, 4], [4, 2],
                                  [8, 4]])
                nc.vector.tensor_copy(dst, src)

                # ---- w_new: context vectors for own samples
                ps_w_full = psB.tile([128, D], f32, tag="gates")
                for g1 in range(2):
                    for tc_i in range(4):
                        for g0 in range(4):
                            j = g0 * 8 + g1 * 4 + tc_i
                            b_g = g1 * 4 + g0
                            nc.tensor.matmul(
                                ps_w_full[32 * g0:32 * g0 + 32, :],
                                awm[:, j * 32:(j + 1) * 32],
                                mm[:, (b_g * 4 + tc_i) * D:
                                   (b_g * 4 + tc_i + 1) * D],
                                start=(g1 == 0 and tc_i == 0),
                                stop=(g1 == 1 and tc_i == 3),
                                tile_position=(0, 32 * g0),
                                skip_group_check=True)
                w_own = wk.tile([128, D], bf16, tag="wown")
                nc.vector.tensor_copy(w_own[:], ps_w_full[:])
                bw_ap = bw_in[:, :]
                for c in range(4):
                    dstw = bass.AP(tensor=bw_ap.tensor,
                                   offset=bw_ap.offset + c * D,
                                   ap=[[4 * D, 2], [1, D]])
                    (nc.sync if c % 2 == 0 else nc.scalar).dma_start(
                        dstw, w_own[32 * c:32 * c + 2, :])
                if not SKIP_CC:
                    nc.gpsimd.collective_compute(
                        "AllGather", ALU.bypass, ins=[bw_in.opt()],
                        outs=[bw_out.opt()], replica_groups=RG)
                else:
                    for cc in range(8):
                        nc.sync.dma_start(bw_out[cc * BL:(cc + 1) * BL, :],
                                          bw_in[:])
                w_g = wk.tile([B, D], bf16, tag="wg2")
                nc.sync.dma_start(w_g[:], bw_out[:])
                ps_wt = psB.tile([128, 256], bf16, tag="misc")
                for c in range(4):
                    nc.tensor.transpose(ps_wt[:, c * B:(c + 1) * B],
                                        w_g[:, c * 128:(c + 1) * 128],
                                        idb[0:B, 0:B])
                nc.vector.tensor_copy(wT[:], ps_wt[:])

                # ---- out projection (off-chain; runs during next AG window)
                ps_o = psB.tile([B, OUT], f32, tag="misc2")
                for c in range(4):
                    nc.tensor.matmul(ps_o[:],
                                     wT[:, c * B:(c + 1) * B],
                                     wp[:, c * OUT:(c + 1) * OUT],
                                     start=(c == 0), stop=False)
                for c in range(8):
                    nc.tensor.matmul(ps_o[:],
                                     hTg[:, c * B:(c + 1) * B],
                                     wp[:, (4 + c) * OUT:(5 + c) * OUT],
                                     start=False, stop=False)
                nc.tensor.matmul(ps_o[:], onb[:], bpv[:],
                                 start=False, stop=True)
                o_sb = wk.tile([B, OUT], f32, tag="osb")
                nc.scalar.copy(o_sb[:], ps_o[:])
                nc.scalar.dma_start(out_d[:, t, :], o_sb[:])

    return nc


# --------------------------------------------------------------- host side
def _prep_inputs(inputs):
    spkr = np.asarray(inputs["spkr"]).astype(np.int64)
    memory = np.asarray(inputs["memory"], np.float32)
    spk_emb = np.asarray(inputs["spkr_emb"], np.float32)
    Wq = np.asarray(inputs["Wq"], np.float32)
    Wm = np.asarray(inputs["Wm"], np.float32)
    Wv = np.asarray(inputs["Wv"], np.float32)
    conv_w = np.asarray(inputs["conv_w"], np.float32)
    loc_w = np.asarray(inputs["loc_w"], np.float32)
    W_ih = np.asarray(inputs["W_ih"], np.float32)
    W_hh = np.asarray(inputs["W_hh"], np.float32)
    b_ih = np.asarray(inputs["b_ih"], np.float32)
    b_hh = np.asarray(inputs["b_hh"], np.float32)
    Wp = np.asarray(inputs["Wp"], np.float32)
    bp = np.asarray(inputs["bp"], np.float32)

    spk_vec = spk_emb[spkr]                       # (B, SPK)
    pm_full = np.einsum("btd,ad->bta", memory, Wm).astype(np.float32)
    G = np.einsum("af,fck->ack", loc_w, conv_w)   # (ATT, 2, KS)
    gmat = np.transpose(G, (1, 2, 0)).reshape(2 * KS, ATT)  # [(c,k), a]

    identf = np.eye(128, dtype=np.float32)
    in_maps = []
    for k in range(NC_N):
        rows = np.concatenate([
            np.arange(128 * k, 128 * (k + 1)),            # i
            1024 + np.arange(128 * k, 128 * (k + 1)),     # f
            3072 + np.arange(128 * k, 128 * (k + 1)),     # o
            2048 + np.arange(128 * k, 128 * (k + 1)),     # g
        ])
        # g block scaled by 2 so a single tanh(0.5*gates) covers all gates
        gsc = np.concatenate([np.ones(384), 2.0 * np.ones(128)])[:, None]
        wcat = np.concatenate([W_ih[rows, :D], 0.5 * W_hh[rows, :]],
                              axis=1) * gsc
        wg_h = np.ascontiguousarray(wcat.T)               # (1536, GS)
        b0 = ((spk_vec @ W_ih[rows, D:D + SPK_DIM].T
               + b_ih[rows] + b_hh[rows]) * gsc.T).astype(ml_dtypes.bfloat16)
        own = slice(BL * k, BL * (k + 1))
        pm_k = np.ascontiguousarray(
            pm_full[own].reshape(BT, ATT).T)              # (ATT, BT)
        ind = np.zeros((B, BT), np.float32)
        for j in range(BL):
            ind[BL * k + j, j * T:(j + 1) * T] = 1.0
        wvoh = np.zeros((ATT, 8 * BL), np.float32)
        for j in range(BL):
            wvoh[:, j * BL + j] = Wv[0]
        wqt = np.ascontiguousarray((0.5 * Wq).T)          # (RNN, ATT)
        wpt = np.concatenate([Wp[:, RNN:], 0.5 * Wp[:, :RNN]],
                             axis=1).T.astype(np.float32)  # (1536, OUT)
        memt = np.ascontiguousarray(
            memory[own].reshape(BL, 4, 128, D).transpose(2, 0, 1, 3)
            .reshape(128, BL * 4 * D))
        in_maps.append({
            "wg": wg_h.astype(ml_dtypes.bfloat16), "b0": b0,
            "pm": pm_k.astype(ml_dtypes.bfloat16),
            "gmat": gmat.astype(ml_dtypes.bfloat16),
            "indic": ind.astype(ml_dtypes.bfloat16),
            "identf": identf, "identb": identf.astype(ml_dtypes.bfloat16),
            "wvoh": wvoh.astype(ml_dtypes.bfloat16),
            "wqt": wqt.astype(ml_dtypes.bfloat16),
            "wpt": wpt.astype(ml_dtypes.bfloat16),
            "bpv": bp.reshape(1, OUT).astype(ml_dtypes.bfloat16),
            "memt": memt.astype(ml_dtypes.bfloat16),
            "onesb": np.ones((1, B), ml_dtypes.bfloat16),
        })
    return in_maps


def kernel(**inputs):
    _install_hooks()
    n_steps = int(np.asarray(inputs["output_timesteps"]))
    if n_steps not in _CACHE:
        _CACHE[n_steps] = _build(n_steps)
    nc = _CACHE[n_steps]
    in_maps = _prep_inputs(inputs)

    from concourse.bass_utils import run_bass_kernel_spmd as _run
    runner = globals().get("run_bass_kernel_spmd", _run)
    res = runner(nc, in_maps, core_ids=list(range(NC_N)))
    global _LAST_RESULT
    _LAST_RESULT = res
    outs = []
    for k in range(NC_N):
        outs.append(res.results[k]["out"][BL * k:BL * (k + 1)])  # (BL, S, OUT)
    full = np.concatenate(outs, axis=0)           # (B, S, OUT)
    return np.ascontiguousarray(full.transpose(0, 2, 1)).astype(np.float32)



# revision 13
# speedup vs baseline: 1.0400x; 1.0400x over previous
"""Trainium2 Bass kernel for nn_Aligner — v3 (pair-pipelined attention,
Shared-space collectives, shortened serial chain).

Sharding: tensor-parallel LSTM gates (512 gate cols/core for all 64 samples)
+ batch-parallel attention (8 samples/core). Two AllGathers per step:
  AG-h: h2 [64,128] per core -> [512,128] Shared; readback via DMA-transpose
        directly into hTg layout (no PE transpose / SBUF staging on chain).
  AG-w: unnormalized context rows + 1/softmax-sum per sample ride together;
        normalization is folded into the post-gather wT transposes by
        multiplying with diag(rs) instead of the identity.
Attention is pipelined per sample-pair: dyn matmul -> tanh -> e -> exp
(unnormalized, accum) -> transposes -> w_new matmuls -> DMA out, so only the
last pair's tail sits on the critical path. Statics (pm copy), gates
bias+h-part, and the output projection float into the AllGather windows.
"""
import json
import os

import numpy as np
import ml_dtypes

import concourse.bass as bass
import concourse.mybir as mybir
import concourse.tile as tile

B, T, D = 64, 512, 512
RNN, ATT, OUT = 1024, 128, 80
SPK_DIM, N_SPKRS = 64, 128
NF, KS = 32, 31
PAD = (KS - 1) // 2
NC_N = 8
BL = B // NC_N            # 8 samples/core
GS = 4 * RNN // NC_N      # 512 gate cols/core
BT = BL * T               # 4096
TPAD = T + 32             # 544 padded alpha row stride
WCOL = 512                # w row payload (normalized pre-send)

f32 = mybir.dt.float32
bf16 = mybir.dt.bfloat16
AF = mybir.ActivationFunctionType
ALU = mybir.AluOpType

_CACHE = {}


def _fix_bir_json(bir):
    """This walrus build allows at most one semaphore wait per instruction;
    hoist extras onto same-engine NoOps inserted just before."""
    j = json.loads(bir)
    n = [0]

    def fix_block(block):
        insts = block.get("instructions")
        if not insts:
            return
        out = []
        for ins in insts:
            waits = (ins.get("sync_info") or {}).get("on_wait") or []
            if len(waits) > 1:
                ins["sync_info"]["on_wait"] = waits[-1:]
                for w in waits[:-1]:
                    n[0] += 1
                    out.append({"engine": ins["engine"], "ins": [], "outs": [],
                                "name": f"I-mwfix-{n[0]}", "opcode": "NoOp",
                                "sync_info": {"on_wait": [w]}})
            out.append(ins)
        block["instructions"] = out

    def walk(o):
        if isinstance(o, dict):
            if isinstance(o.get("instructions"), list):
                fix_block(o)
            for v in o.values():
                walk(v)
        elif isinstance(o, list):
            for v in o:
                walk(v)

    walk(j)
    return json.dumps(j).encode()


def _install_hooks():
    if getattr(_install_hooks, "done", False):
        return
    _install_hooks.done = True
    import concourse.bass_utils as bu
    import concourse.bass2jax as b2j
    orig = bu.compile_bir_kernel

    def patched(bir_json, tmpdir, neff_name="file.neff"):
        if isinstance(bir_json, str):
            bir_json = bir_json.encode()
        return orig(_fix_bir_json(bir_json), tmpdir, neff_name=neff_name)

    bu.compile_bir_kernel = patched
    b2j.compile_bir_kernel = patched


def _build(n_steps):
    SKIP_CC = os.environ.get("SKIP_CC", "0") == "1"

    nc = bass.Bass("TRN2", target_bir_lowering=False, debug=False,
                   num_devices=NC_N)

    def din(name, shape, dt=f32):
        return nc.dram_tensor(name, shape, dt, kind="ExternalInput")

    x_wg = din("wg", [12 * 128, GS], bf16)
    x_b0 = din("b0", [B, GS], bf16)
    x_pm = din("pm", [ATT, BT], bf16)
    x_g = din("gmat", [62, ATT], bf16)
    x_ind = din("indic", [B, BT], bf16)     # rows: global sample one-hots
    x_idb = din("identb", [128, 128], bf16)
    x_wv = din("wvcol", [ATT, 4], bf16)
    x_wq = din("wqt", [RNN, ATT], bf16)
    x_wp = din("wpt", [12 * 128, OUT], bf16)
    x_bp = din("bpv", [1, OUT], bf16)
    x_on = din("onesb", [1, B], bf16)
    x_mm = din("memt", [128, BL * 4 * D], bf16)
    out_d = nc.dram_tensor("out", [B, n_steps, OUT], f32, kind="ExternalOutput")

    RG = [list(range(NC_N))]

    with tile.TileContext(nc) as tc:
        with (
            tc.tile_pool(name="const", bufs=1) as cst,
            tc.tile_pool(name="state", bufs=1) as st,
            tc.tile_pool(name="work", bufs=2) as wk,
            tc.tile_pool(name="psA", bufs=2, space="PSUM") as psA,
            tc.tile_pool(name="psB", bufs=1, space="PSUM") as psB,
            tc.tile_pool(name="dram", bufs=1, space="DRAM") as dram,
        ):
            # ---------------- constants
            wg = cst.tile([128, 12 * GS], bf16)
            for c in range(12):
                nc.sync.dma_start(wg[:, c * GS:(c + 1) * GS],
                                  x_wg[c * 128:(c + 1) * 128, :])
            b0 = cst.tile([B, GS], bf16)
            nc.sync.dma_start(b0[:], x_b0[:])
            pm = cst.tile([ATT, BT], bf16)
            nc.sync.dma_start(pm[:], x_pm[:])
            idb = cst.tile([128, 128], bf16)
            nc.sync.dma_start(idb[:], x_idb[:])
            wv = cst.tile([ATT, 4], bf16)
            nc.sync.dma_start(wv[:], x_wv[:])
            wq = cst.tile([128, 8 * ATT], bf16)
            for c in range(8):
                nc.sync.dma_start(wq[:, c * ATT:(c + 1) * ATT],
                                  x_wq[c * 128:(c + 1) * 128, :])
            wp = cst.tile([128, 12 * OUT], bf16)
            for c in range(12):
                nc.sync.dma_start(wp[:, c * OUT:(c + 1) * OUT],
                                  x_wp[c * 128:(c + 1) * 128, :])
            bpv = cst.tile([1, OUT], bf16)
            nc.sync.dma_start(bpv[:], x_bp[:])
            onb = cst.tile([1, B], bf16)
            nc.sync.dma_start(onb[:], x_on[:])
            mm = cst.tile([128, BL * 4 * D], bf16)
            nc.sync.dma_start(mm[:], x_mm[:])

            # ---------------- state
            hTg_bufs = [st.tile([128, 8 * B], bf16, name=f"hTg{i}")
                        for i in range(2)]
            wT_bufs = [st.tile([128, 4 * B], bf16, name=f"wT{i}")
                       for i in range(2)]
            c_st = st.tile([B, 128], f32)
            a_cum = st.tile([BL, T], f32)
            af = st.tile([2, BL * TPAD], bf16)
            awm = st.tile([128, 32 * 32], bf16)
            th = st.tile([ATT, BT], bf16)
            aw_b = st.tile([BL, T], bf16)
            rs8 = st.tile([BL, 1], f32)
            # fused conv operands: lhsT rows 0:64 pq (dynamic), 64:126 G
            conv_lhs = st.tile([126, ATT], bf16)
            nc.sync.dma_start(conv_lhs[B:126, :], x_g[:])
            conv_rhs = st.tile([126, BT], bf16)
            nc.sync.dma_start(conv_rhs[0:B, :], x_ind[:])
            for t_ in (c_st, a_cum, af, awm, aw_b, rs8):
                nc.vector.memset(t_[:], 0.0)
            nc.vector.memset(th[:], 0.0)
            for t_ in hTg_bufs + wT_bufs:
                nc.vector.memset(t_[:], 0.0)
            nc.vector.memset(conv_rhs[B:126, :], 0.0)
            nc.vector.memset(conv_lhs[0:B, :], 0.0)

            # ---------------- collective bounce buffers
            bh_in = dram.tile([B, 128], bf16)          # h2 rows (untransposed)
            bw_in = dram.tile([BL, WCOL], bf16)        # w rows + rs col
            baf = dram.tile([2, BL * TPAD], bf16)

            # ---------------- steps
            for t in range(n_steps):
                hTg_p = hTg_bufs[(t + 1) % 2]   # h(t-1) gathered
                hTg = hTg_bufs[t % 2]           # h(t) gathered (this step)
                wT_p = wT_bufs[(t + 1) % 2]     # w(t-1) gathered
                wT = wT_bufs[t % 2]             # w(t) gathered (this step)
                # ---- gates: bias + h-part (floats into AG-w(t-1) window)
                ps_g = psB.tile([B, GS], f32, tag="gates")
                nc.tensor.matmul(ps_g[:], idb[0:B, 0:B], b0[:],
                                 start=True, stop=False)
                for c in range(8):
                    nc.tensor.matmul(ps_g[:], hTg_p[:, c * B:(c + 1) * B],
                                     wg[:, (4 + c) * GS:(5 + c) * GS],
                                     start=False, stop=False)
                # ---- gates: w-part (chain: waits on wT(t-1))
                for c in range(4):
                    nc.tensor.matmul(ps_g[:], wT_p[:, c * B:(c + 1) * B],
                                     wg[:, c * GS:(c + 1) * GS],
                                     start=False, stop=(c == 3))

                # ---- pointwise LSTM (g-weights pre-scaled 2x on host):
                # yifog = tanh(0.5*gates): cols 0:384 -> 2*sig(x)-1, else tanh
                yifog = wk.tile([B, 512], f32, tag="yifog")
                nc.scalar.activation(yifog[:], ps_g[:], AF.Tanh, scale=0.5)
                t1 = wk.tile([B, 128], f32, tag="t1")
                t2 = wk.tile([B, 128], f32, tag="t2")
                # t1 = (f'+1)*c ; t2 = (i'+1)*tanh(g) ; t1+t2 = 2*c_new
                nc.vector.scalar_tensor_tensor(
                    t1[:], yifog[:, 128:256], 1.0, c_st[:], ALU.add, ALU.mult)
                nc.vector.scalar_tensor_tensor(
                    t2[:], yifog[:, 0:128], 1.0, yifog[:, 384:512],
                    ALU.add, ALU.mult)
                nc.vector.tensor_add(t1[:], t1[:], t2[:])
                tct = wk.tile([B, 128], f32, tag="tct")
                nc.scalar.activation(tct[:], t1[:], AF.Tanh, scale=0.5)
                h2 = wk.tile([B, 128], bf16, tag="h2")
                nc.vector.scalar_tensor_tensor(
                    h2[:], yifog[:, 256:384], 1.0, tct[:], ALU.add, ALU.mult)
                nc.vector.tensor_scalar_mul(c_st[:], t1[:], 0.5)  # off-chain

                # ---- h2 -> DRAM -> AllGather -> transpose-DMA -> hTg
                nc.sync.dma_start(bh_in[:], h2[:])
                bho = dram.tile([NC_N * B, 128], bf16, name=f"bho{t}",
                                addr_space=("Local" if SKIP_CC else "Shared"))
                if not SKIP_CC:
                    nc.gpsimd.collective_compute(
                        "AllGather", ALU.bypass, ins=[bh_in.opt()],
                        outs=[bho.opt()], replica_groups=RG)
                else:
                    for cc in range(8):
                        nc.sync.dma_start(bho[cc * B:(cc + 1) * B, :],
                                          bh_in[:])
                nc.sync.dma_start_transpose(hTg[:, :], bho[:, :])

                # ---- pq (all 64 samples) -> bf16 lhsT rows of conv_lhs
                ps_pq = psB.tile([B, ATT], f32, tag="gates")
                for c in range(8):
                    nc.tensor.matmul(ps_pq[:],
                                     hTg[:, c * B:(c + 1) * B],
                                     wq[:, c * ATT:(c + 1) * ATT],
                                     start=(c == 0), stop=(c == 7))
                nc.vector.tensor_copy(conv_lhs[0:B, :], ps_pq[:])

                # ---- attention, pipelined per sample pair q: samples 2q,2q+1
                ps_w = psB.tile([128, D], f32, tag="wnew")
                w_sb = wk.tile([2, 4 * D], bf16, tag="wsb")
                for q in range(4):
                    ps_q = psA.tile([ATT, 2 * T], f32, tag="argq")
                    for c in range(2):
                        lo = c * 512
                        g_lo = 2 * q * T + lo
                        nc.tensor.matmul(ps_q[:, lo:lo + 512], idb[:, 0:ATT],
                                         pm[:, g_lo:g_lo + 512],
                                         start=True, stop=False)
                        nc.tensor.matmul(ps_q[:, lo:lo + 512], conv_lhs[:],
                                         conv_rhs[:, g_lo:g_lo + 512],
                                         start=False, stop=True)
                    nc.scalar.activation(th[:, 2 * q * T:(2 * q + 2) * T],
                                         ps_q[:], AF.Tanh)
                    ps_e = psB.tile([2, T], f32, tag="e")
                    for g1 in range(2):
                        s = 2 * q + g1
                        nc.tensor.matmul(ps_e[:], wv[:, 2 * g1:2 * g1 + 2],
                                         th[:, s * T:(s + 1) * T],
                                         start=(g1 == 0), stop=(g1 == 1))
                    # unnormalized softmax numerator + per-sample sum
                    aw_p = wk.tile([2, T], bf16, tag=f"awp{q % 2}")
                    s_p = wk.tile([2, 1], f32, tag=f"sp{q % 2}")
                    nc.scalar.activation(aw_p[:], ps_e[:], AF.Exp,
                                         accum_out=s_p[:])
                    rs_p = wk.tile([2, 1], f32, tag=f"rsp{q % 2}")
                    nc.vector.reciprocal(rs_p[:], s_p[:])
                    nc.sync.dma_start(aw_b[2 * q:2 * q + 2, :], aw_p[:])
                    nc.sync.dma_start(rs8[2 * q:2 * q + 2, :], rs_p[:])
                    ps_at = psB.tile([128, 8], bf16, tag="misc")
                    for tc_i in range(4):
                        nc.tensor.transpose(
                            ps_at[:, tc_i * 2:tc_i * 2 + 2],
                            aw_p[:, tc_i * 128:(tc_i + 1) * 128],
                            idb[0:2, 0:2])
                    # scatter into block-diag lhsT blocks j=(2q+g1)*4+tc
                    awm_ap = awm[:, :]
                    pa_ap = ps_at[:, :]
                    dst = bass.AP(tensor=awm_ap.tensor,
                                  offset=awm_ap.offset + 256 * q,
                                  ap=[[awm_ap.ap[0][0], 128], [129, 2],
                                      [32, 4]])
                    src = bass.AP(tensor=pa_ap.tensor, offset=pa_ap.offset,
                                  ap=[[pa_ap.ap[0][0], 128], [1, 2], [2, 4]])
                    nc.vector.tensor_copy(dst, src)
                    # w_new for this pair (unnormalized)
                    for g1 in range(2):
                        for tc_i in range(4):
                            j = (2 * q + g1) * 4 + tc_i
                            nc.tensor.matmul(
                                ps_w[32 * q:32 * q + 32, :],
                                awm[:, j * 32:(j + 1) * 32],
                                mm[:, j * D:(j + 1) * D],
                                start=(g1 == 0 and tc_i == 0),
                                stop=(g1 == 1 and tc_i == 3),
                                tile_position=(0, 32 * q),
                                skip_group_check=True)
                    nc.vector.tensor_scalar_mul(
                        w_sb[:, q * D:(q + 1) * D],
                        ps_w[32 * q:32 * q + 2, :], rs_p[:])
                    nc.sync.dma_start(bw_in[2 * q:2 * q + 2, :],
                                      w_sb[:, q * D:(q + 1) * D])

                # ---- local normalized alpha bookkeeping (off-chain)
                aw_n = wk.tile([BL, T], bf16, tag="awn")
                nc.vector.tensor_scalar_mul(aw_n[:], aw_b[:], rs8[:])
                nc.vector.scalar_tensor_tensor(
                    a_cum[:], aw_b[:], rs8[:], a_cum[:], ALU.mult, ALU.add)
                acb = wk.tile([BL, T], bf16, tag="acb")
                nc.vector.tensor_copy(acb[:], a_cum[:])

                # ---- alpha-flat rows + im2col for next step's conv statics
                af_r = af[:, :].rearrange("c (b p) -> c b p", b=BL)
                nc.scalar.dma_start(af_r[0:1, :, 16:16 + T], aw_n[:])
                nc.scalar.dma_start(af_r[1:2, :, 16:16 + T], acb[:])
                nc.gpsimd.dma_start(baf[:], af[:])
                baf_ap = baf[:, :]
                for ci in range(2):
                    for hh in range(2):
                        k0 = hh * 16
                        nk = 16 if hh == 0 else KS - 16
                        src = bass.AP(
                            tensor=baf_ap.tensor,
                            offset=baf_ap.offset + ci * BL * TPAD + k0 + 1,
                            ap=[[1, nk], [TPAD, BL], [1, T]])
                        nc.gpsimd.dma_start(
                            conv_rhs[B + ci * KS + k0:
                                     B + ci * KS + k0 + nk, :], src)

                # ---- AllGather w (unnormalized) + rs
                bwo = dram.tile([B, WCOL], bf16, name=f"bwo{t}",
                                addr_space=("Local" if SKIP_CC else "Shared"))
                if not SKIP_CC:
                    nc.gpsimd.collective_compute(
                        "AllGather", ALU.bypass, ins=[bw_in.opt()],
                        outs=[bwo.opt()], replica_groups=RG)
                else:
                    for cc in range(8):
                        nc.sync.dma_start(bwo[cc * BL:(cc + 1) * BL, :],
                                          bw_in[:])
                w_g = wk.tile([B, WCOL], bf16, tag="wg2")
                nc.sync.dma_start(w_g[:], bwo[:])
                ps_wt = psB.tile([128, 256], bf16, tag="misc")
                for c in range(4):
                    nc.tensor.transpose(ps_wt[:, c * B:(c + 1) * B],
                                        w_g[:, c * 128:(c + 1) * 128],
                                        idb[0:B, 0:B])
                nc.vector.tensor_copy(wT[:], ps_wt[:])

                # ---- out projection (h-parts first so they float early)
                ps_o = psB.tile([B, OUT], f32, tag="e")
                nc.tensor.matmul(ps_o[:], onb[:], bpv[:],
                                 start=True, stop=False)
                for c in range(8):
                    nc.tensor.matmul(ps_o[:],
                                     hTg[:, c * B:(c + 1) * B],
                                     wp[:, (4 + c) * OUT:(5 + c) * OUT],
                                     start=False, stop=False)
                for c in range(4):
                    nc.tensor.matmul(ps_o[:],
                                     wT[:, c * B:(c + 1) * B],
                                     wp[:, c * OUT:(c + 1) * OUT],
                                     start=False, stop=(c == 3))
                o_sb = wk.tile([B, OUT], f32, tag="osb")
                nc.scalar.copy(o_sb[:], ps_o[:])
                nc.scalar.dma_start(out_d[:, t, :], o_sb[:])

    return nc


# --------------------------------------------------------------- host side
def _prep_inputs(inputs):
    spkr = np.asarray(inputs["spkr"]).astype(np.int64)
    memory = np.asarray(inputs["memory"], np.float32)
    spk_emb = np.asarray(inputs["spkr_emb"], np.float32)
    Wq = np.asarray(inputs["Wq"], np.float32)
    Wm = np.asarray(inputs["Wm"], np.float32)
    Wv = np.asarray(inputs["Wv"], np.float32)
    conv_w = np.asarray(inputs["conv_w"], np.float32)
    loc_w = np.asarray(inputs["loc_w"], np.float32)
    W_ih = np.asarray(inputs["W_ih"], np.float32)
    W_hh = np.asarray(inputs["W_hh"], np.float32)
    b_ih = np.asarray(inputs["b_ih"], np.float32)
    b_hh = np.asarray(inputs["b_hh"], np.float32)
    Wp = np.asarray(inputs["Wp"], np.float32)
    bp = np.asarray(inputs["bp"], np.float32)

    spk_vec = spk_emb[spkr]                       # (B, SPK)
    pm_full = np.einsum("btd,ad->bta", memory, Wm).astype(np.float32)
    G = np.einsum("af,fck->ack", loc_w, conv_w)   # (ATT, 2, KS)
    gmat = np.transpose(G, (1, 2, 0)).reshape(2 * KS, ATT)  # [(c,k), a]

    identf = np.eye(128, dtype=np.float32)
    wv4 = np.zeros((ATT, 4), np.float32)
    wv4[:, 0] = Wv[0]
    wv4[:, 3] = Wv[0]
    in_maps = []
    for k in range(NC_N):
        rows = np.concatenate([
            np.arange(128 * k, 128 * (k + 1)),            # i
            1024 + np.arange(128 * k, 128 * (k + 1)),     # f
            3072 + np.arange(128 * k, 128 * (k + 1)),     # o
            2048 + np.arange(128 * k, 128 * (k + 1)),     # g
        ])
        # g block scaled by 2 so a single tanh(0.5*gates) covers all gates
        gsc = np.concatenate([np.ones(384), 2.0 * np.ones(128)])[:, None]
        wcat = np.concatenate([W_ih[rows, :D], 0.5 * W_hh[rows, :]],
                              axis=1) * gsc
        wg_h = np.ascontiguousarray(wcat.T)               # (1536, GS)
        b0 = ((spk_vec @ W_ih[rows, D:D + SPK_DIM].T
               + b_ih[rows] + b_hh[rows]) * gsc.T).astype(ml_dtypes.bfloat16)
        own = slice(BL * k, BL * (k + 1))
        pm_k = np.ascontiguousarray(
            pm_full[own].reshape(BT, ATT).T)              # (ATT, BT)
        ind = np.zeros((B, BT), np.float32)
        for j in range(BL):
            ind[BL * k + j, j * T:(j + 1) * T] = 1.0
        wqt = np.ascontiguousarray((0.5 * Wq).T)          # (RNN, ATT)
        wpt = np.concatenate([Wp[:, RNN:], 0.5 * Wp[:, :RNN]],
                             axis=1).T.astype(np.float32)  # (1536, OUT)
        memt = np.ascontiguousarray(
            memory[own].reshape(BL, 4, 128, D).transpose(2, 0, 1, 3)
            .reshape(128, BL * 4 * D))
        in_maps.append({
            "wg": wg_h.astype(ml_dtypes.bfloat16), "b0": b0,
            "pm": pm_k.astype(ml_dtypes.bfloat16),
            "gmat": gmat.astype(ml_dtypes.bfloat16),
            "indic": ind.astype(ml_dtypes.bfloat16),
            "identb": identf.astype(ml_dtypes.bfloat16),
            "wvcol": wv4.astype(ml_dtypes.bfloat16),
            "wqt": wqt.astype(ml_dtypes.bfloat16),
            "wpt": wpt.astype(ml_dtypes.bfloat16),
            "bpv": bp.reshape(1, OUT).astype(ml_dtypes.bfloat16),
            "memt": memt.astype(ml_dtypes.bfloat16),
            "onesb": np.ones((1, B), ml_dtypes.bfloat16),
        })
    return in_maps


def kernel(**inputs):
    _install_hooks()
    n_steps = int(np.asarray(inputs["output_timesteps"]))
    if n_steps not in _CACHE:
        _CACHE[n_steps] = _build(n_steps)
    nc = _CACHE[n_steps]
    in_maps = _prep_inputs(inputs)

    from concourse.bass_utils import run_bass_kernel_spmd as _run
    runner = globals().get("run_bass_kernel_spmd", _run)
    res = runner(nc, in_maps, core_ids=list(range(NC_N)))
    global _LAST_RESULT
    _LAST_RESULT = res
    outs = []
    for k in range(NC_N):
        outs.append(res.results[k]["out"][BL * k:BL * (k + 1)])  # (BL, S, OUT)
    full = np.concatenate(outs, axis=0)           # (B, S, OUT)
    return np.ascontiguousarray(full.transpose(0, 2, 1)).astype(np.float32)
